# revision 23
# baseline (speedup 1.0000x reference)
"""nn_BlockSharedRounding Trainium2 kernel.

Computes the forward of the block-shared soft rounding reference:
    a   = |x| + 0.5*tanh(delta_raw) per 32-block
    ord = searchsorted(BOUNDS, a, 'left')
    q   = VALUES[ord]                       (== abs_mix forward value)

Strategy: data-parallel over 8 NeuronCores (rows of x). Per core, a raw
Bass kernel streams [128, fd] fp32 chunks through ONE fused custom DVE
op that evaluates the e2m1 ordinal as the (round-to-nearest) uint8
conversion of a min/max of three linear functions of
a' = |x| + d' (d' = 0.5*tanh(delta)+1.5):

    byte = rne( min( a' + min(a', 3.5), max(a' + 3.0, 3.0^2) ) )

which equals ord+3 for ord in 0..7 (bytes >= 10 encode ord 7; bytes <= 3
encode ord 0).  The host decodes both outputs from the single uint8
stream with 256-entry LUTs: ord = clip(byte-3, 0, 7), q = VALUES[ord].

Per-core HBM traffic is 16.5 MiB read + 4 MiB write (vs 29.9 MB for the
4-pass predecessor), and the vector engine does a single 8-stage pass
(34 us) instead of four, leaving the kernel DMA-bound at ~60 us.

The fused op only has two scalar slots (the d' broadcast occupies Src1
in TTSS encoding), so the high-region cap constant is derived in the
hoisted seed stage as SQUARE(C0): 2.75^2 = 7.5625, which lies in the
valid cap window [6.5+B, 7+B) for byte base B=1.
"""
import numpy as np

import concourse.bass as bass
import concourse.bacc as bacc
import concourse.mybir as mybir
import concourse.dve_ops as DO
from concourse.dve_uop import DveOpSpec
from concourse.dve_spec import (
    Spec, Src0, Src1, C0, C1, Zero, Bin, AluOp, lower, minn, maxx, sq,
    _has_src1,
)
from concourse.bass_utils import run_bass_kernel_spmd

# ---------------------------------------------------------------- constants
N_CORES = 8
ROWS, COLS = 4096, 8192
SHARD_ROWS = ROWS // N_CORES            # 512
SHARD_ELEMS = SHARD_ROWS * COLS         # 4,194,304
BLOCK = 32
FD = 8192                               # max free dim per chunk
CHUNK_FDS = [8192, 8192, 8192, 4096, 2048, 1536, 512]
assert sum(CHUNK_FDS) * 128 == SHARD_ELEMS
PACK4 = False                           # pack two 4-bit ordinals per byte

# rne-mode design: byte = ord + 3, d' offset 1.5
D_OFFSET = 1.5
S_MIN = 3.5         # C0: region-B line constant (min operand)
S_ADD = 3.0         # C1: region-C line additive; squared -> high-region cap

VALUES = np.array([0.0, 0.5, 1.0, 1.5, 2.0, 3.0, 4.0, 6.0], dtype=np.float32)
_ORD_LUT = np.clip(np.arange(256) - 3, 0, 7).astype(np.int32)
_Q_LUT = VALUES[_ORD_LUT]
# 4-bit pack decode: lo nibble = even element, hi nibble = odd element
_LO_ORD = np.clip((np.arange(256) & 15) - 3, 0, 7).astype(np.int32)
_HI_ORD = np.clip((np.arange(256) >> 4) - 3, 0, 7).astype(np.int32)
_LO_Q = VALUES[_LO_ORD]
_HI_Q = VALUES[_HI_ORD]

# ---------------------------------------------------------------- custom op
def _register_op(name, spec, subdim=False):
    if name in DO._SUB_OPCODE_FOR_NAME:          # idempotent across re-imports
        return next(op for op in DO.OPS if op.name == name)
    row = DO._CUSTOM_DVE_ROW_BASE + len(DO.OPS)
    shas = {}
    for ver in ("v3", "v4"):
        sc = DveOpSpec(name=name, opcode=row, uops=lower(spec, ver=ver),
                       rd1_en=_has_src1(spec))
        shas[ver] = sc.sha(ver)
    op = DO.DveOp(name, spec, subdim=subdim, uops_sha=shas)
    DO.OPS.append(op)
    DO._SUB_OPCODE_FOR_NAME[name] = row
    return op


def _absn(x):
    return Bin(AluOp.ABSOLUTE_VALUE, x, Zero)


def _fused_ref(in0, in1, s0, s1, imm2):
    a = np.abs(in0) + in1
    return np.minimum(a + np.minimum(a, s0),
                      np.maximum(a + s1, s1 * s1)).astype(np.float32)


_a = _absn(Src0) + Src1
P_FUSED = _register_op("BSR_FUSED_ORD_R", Spec(
    body=minn(_a + minn(_a, C0), maxx(_a + C1, sq(C1))),
    reference=_fused_ref,
))

# out[k] = min(in0[2k],15) + 16*min(in0[2k+1],15)  (via strided in0/in1 APs)
P_PACK = _register_op("BSR_PACK4", Spec(
    body=minn(Src0, C1) + minn(Src1, C1) * C0,
    reference=lambda in0, in1, s0, s1, imm2: (
        np.minimum(in0, s1) + np.minimum(in1, s1) * s0).astype(np.float32),
))

# ---------------------------------------------------------------- bass module
_NC_CACHE = {}


def _ap(t, offset, ap):
    return bass.AP(tensor=getattr(t, "tensor", t), offset=offset, ap=ap)


def build_nc():
    if "nc" in _NC_CACHE:
        return _NC_CACHE["nc"]
    nc = bacc.Bacc(None, target_bir_lowering=False)
    x = nc.dram_tensor("x", [SHARD_ELEMS], mybir.dt.float32, kind="ExternalInput")
    d = nc.dram_tensor("d", [SHARD_ELEMS // BLOCK], mybir.dt.float32,
                       kind="ExternalInput")
    o_elems = SHARD_ELEMS // 2 if PACK4 else SHARD_ELEMS
    o = nc.dram_tensor("o", [o_elems], mybir.dt.uint8, kind="ExternalOutput")

    DBMAX = FD // BLOCK
    NSLOT = 3
    xs = [nc.alloc_sbuf_tensor(f"xs{s}", [128, FD], mybir.dt.float32).ap()
          for s in range(NSLOT)]
    ds = [nc.alloc_sbuf_tensor(f"ds{s}", [128, DBMAX], mybir.dt.float32).ap()
          for s in range(NSLOT)]
    os_ = [nc.alloc_sbuf_tensor(f"os{s}", [128, FD], mybir.dt.uint8).ap()
           for s in range(NSLOT)]
    ps = [nc.alloc_sbuf_tensor(f"ps{s}", [128, FD // 2], mybir.dt.uint8).ap()
          for s in range(NSLOT)] if PACK4 else os_

    offs = [0]
    for f in CHUNK_FDS:
        offs.append(offs[-1] + 128 * f)
    NCH = len(CHUNK_FDS)

    # Per-slot load/store sems: DMA completions from different chunks land
    # out of order, so one shared counter would release a consumer while the
    # current chunk's transfer is still in flight. Within one slot, chunks
    # are NSLOT apart and the pipeline (wsem gates) guarantees ordering.
    with (
        nc.semaphore("ldsem0") as ldsem0,
        nc.semaphore("ldsem1") as ldsem1,
        nc.semaphore("ldsem2") as ldsem2,
        nc.semaphore("stsem0") as stsem0,
        nc.semaphore("stsem1") as stsem1,
        nc.semaphore("stsem2") as stsem2,
        nc.semaphore("fsem") as fsem,     # fused-op completions (frees xs/ds)
        nc.semaphore("wsem") as wsem,     # chunk-done (last DVE op) completions
        nc.Block() as block,
    ):
        ldsem = [ldsem0, ldsem1, ldsem2]
        stsem = [stsem0, stsem1, stsem2]
        xfree = fsem if PACK4 else wsem

        @block.sync
        def _(sync):
            # loads only: a store's wsem wait must never block load issue
            for i in range(NCH):
                s = i % NSLOT
                fd = CHUNK_FDS[i]
                db = fd // BLOCK
                if i >= NSLOT:
                    # slot's previous op consumed xs/ds
                    sync.wait_ge(xfree, i - NSLOT + 1)
                sync.dma_start(
                    out=xs[s][:, :fd],
                    in_=_ap(x, offs[i], [[fd, 128], [1, fd]]),
                ).then_inc(ldsem[s], 16)
                sync.dma_start(
                    out=ds[s][:, :db],
                    in_=_ap(d, offs[i] // BLOCK, [[db, 128], [1, db]]),
                ).then_inc(ldsem[s], 16)

        @block.scalar
        def _(scalar):
            # stores on the Act engine's HWDGE ring (qActDynamicHW): parallel
            # to sync's ring, and HW descriptor generation (gpsimd would be
            # the slow SWDGE path). Gated only by op completions.
            for j in range(NCH):
                s = j % NSLOT
                fd = CHUNK_FDS[j]
                scalar.wait_ge(wsem, j + 1)
                if PACK4:
                    scalar.dma_start(
                        out=_ap(o, offs[j] // 2, [[fd // 2, 128], [1, fd // 2]]),
                        in_=ps[s][:, :fd // 2],
                    ).then_inc(stsem[s], 16)
                else:
                    scalar.dma_start(
                        out=_ap(o, offs[j], [[fd, 128], [1, fd]]),
                        in_=os_[s][:, :fd],
                    ).then_inc(stsem[s], 16)
            for s in range(NSLOT):
                n_st = len([i for i in range(NCH) if i % NSLOT == s])
                scalar.wait_ge(stsem[s], 16 * n_st)

        @block.vector
        def _(vector):
            for i in range(NCH):
                s = i % NSLOT
                fd = CHUNK_FDS[i]
                db = fd // BLOCK
                vector.wait_ge(ldsem[s], 32 * (i // NSLOT + 1))
                if not PACK4 and i >= NSLOT:
                    # slot's previous store must have drained os_
                    vector.wait_ge(stsem[s], 16 * (i // NSLOT))
                fused = nc.vector._custom_dve(
                    P_FUSED,
                    out=_ap(os_[s], 0, [os_[s].ap[0], [BLOCK, db], [1, BLOCK]]),
                    in0=_ap(xs[s], 0, [xs[s].ap[0], [BLOCK, db], [1, BLOCK]]),
                    in1=_ap(ds[s], 0, [ds[s].ap[0], [1, db], [0, BLOCK]]),
                    s0=S_MIN, s1=S_ADD,
                )
                if PACK4:
                    fused.then_inc(fsem, 1)
                    if i >= NSLOT:
                        # slot's previous store must have drained ps
                        vector.wait_ge(stsem[s], 16 * (i // NSLOT))
                    nc.vector._custom_dve(
                        P_PACK,
                        out=ps[s][:, :fd // 2],
                        in0=_ap(os_[s], 0, [os_[s].ap[0], [2, fd // 2]]),
                        in1=_ap(os_[s], 1, [os_[s].ap[0], [2, fd // 2]]),
                        s0=16.0, s1=15.0,
                    ).then_inc(wsem, 1)
                else:
                    fused.then_inc(wsem, 1)

    nc.compile()
    _NC_CACHE["nc"] = nc
    return nc


# ---------------------------------------------------------------- host entry
def _delta_device(delta_raw):
    """0.5*tanh + D_OFFSET on the default jax backend — tanh bit-matches the
    oracle's eager computation (backend tanh differs from libm)."""
    import jax.numpy as jnp
    return np.asarray(0.5 * jnp.tanh(jnp.asarray(np.asarray(delta_raw)))
                      + np.float32(D_OFFSET))


def _install_trace_shim():
    """Optional: register the axon NTFF profiling hook so _trace=True works
    in containers whose antenv lacks axon_hooks. No-op on failure."""
    import sys, types
    if "antenv.axon_hooks" in sys.modules:
        return
    try:
        from trn_agent_boot.trn_boot import _ntff_profile_via_ctypes
        hook = _ntff_profile_via_ctypes("/opt/axon/libaxon_pjrt.so")
        mod = types.ModuleType("antenv.axon_hooks")
        mod.get_axon_ntff_profile_hook = lambda: hook
        mod.set_axon_ntff_profile_hook = lambda h: None
        sys.modules["antenv.axon_hooks"] = mod
    except Exception:
        pass


def kernel(x_scaled, delta_raw, _trace=False):
    if _trace:
        _install_trace_shim()
    x_scaled = np.ascontiguousarray(np.asarray(x_scaled), dtype=np.float32)
    delta = _delta_device(delta_raw).astype(np.float32, copy=False)

    nc = build_nc()
    in_maps = []
    for c in range(N_CORES):
        xsh = x_scaled[c * SHARD_ROWS:(c + 1) * SHARD_ROWS].reshape(-1)
        dsh = delta[c * (SHARD_ELEMS // BLOCK):(c + 1) * (SHARD_ELEMS // BLOCK)]
        in_maps.append({"x": xsh, "d": np.ascontiguousarray(dsh)})

    res = run_bass_kernel_spmd(nc, in_maps, list(range(N_CORES)), trace=_trace)

    ob = np.concatenate([res.results[c]["o"] for c in range(N_CORES)])
    if PACK4:
        o = np.empty(ROWS * COLS, dtype=np.int32)
        q = np.empty(ROWS * COLS, dtype=np.float32)
        o[0::2] = _LO_ORD[ob]
        o[1::2] = _HI_ORD[ob]
        q[0::2] = _LO_Q[ob]
        q[1::2] = _HI_Q[ob]
        o = o.reshape(ROWS, COLS)
        q = q.reshape(ROWS, COLS)
    else:
        o = _ORD_LUT[ob].reshape(ROWS, COLS)
        q = _Q_LUT[ob].reshape(ROWS, COLS)
    out = (q, o)
    if _trace:
        return out, res
    return out


# revision 24
# speedup vs baseline: 1.1541x; 1.1541x over previous
"""nn_BlockSharedRounding Trainium2 kernel.

Computes the forward of the block-shared soft rounding reference:
    a   = |x| + 0.5*tanh(delta_raw) per 32-block
    ord = searchsorted(BOUNDS, a, 'left')
    q   = VALUES[ord]                       (== abs_mix forward value)

Strategy: data-parallel over 8 NeuronCores (rows of x). Per core, a raw
Bass kernel streams [128, fd] fp32 chunks through ONE fused custom DVE
op that evaluates the e2m1 ordinal as the (round-to-nearest) uint8
conversion of a min/max of three linear functions of
a' = |x| + d' (d' = 0.5*tanh(delta)+1.5):

    byte = rne( min( a' + min(a', 3.5), max(a' + 3.0, 3.0^2) ) )

which equals ord+3 for ord in 0..7 (bytes >= 10 encode ord 7; bytes <= 3
encode ord 0).  The host decodes both outputs from the single uint8
stream with 256-entry LUTs: ord = clip(byte-3, 0, 7), q = VALUES[ord].

Per-core HBM traffic is 16.5 MiB read + 4 MiB write (vs 29.9 MB for the
4-pass predecessor), and the vector engine does a single 8-stage pass
(34 us) instead of four, leaving the kernel DMA-bound at ~60 us.

The fused op only has two scalar slots (the d' broadcast occupies Src1
in TTSS encoding), so the high-region cap constant is derived in the
hoisted seed stage as SQUARE(C0): 2.75^2 = 7.5625, which lies in the
valid cap window [6.5+B, 7+B) for byte base B=1.
"""
import numpy as np

import concourse.bass as bass
import concourse.bacc as bacc
import concourse.mybir as mybir
import concourse.dve_ops as DO
from concourse.dve_uop import DveOpSpec
from concourse.dve_spec import (
    Spec, Src0, Src1, C0, C1, Zero, Bin, AluOp, lower, minn, maxx, sq,
    _has_src1,
)
from concourse.bass_utils import run_bass_kernel_spmd

# ---------------------------------------------------------------- constants
N_CORES = 8
ROWS, COLS = 4096, 8192
SHARD_ROWS = ROWS // N_CORES            # 512
SHARD_ELEMS = SHARD_ROWS * COLS         # 4,194,304
BLOCK = 32
FD = 8192                               # max free dim per chunk
CHUNK_FDS = [8192, 8192, 8192, 4096, 2048, 1536, 512]
assert sum(CHUNK_FDS) * 128 == SHARD_ELEMS
PACK4 = False                           # pack two 4-bit ordinals per byte

# rne-mode design: byte = ord + 3, d' offset 1.5
D_OFFSET = 1.5
S_MIN = 3.5         # C0: region-B line constant (min operand)
S_ADD = 3.0         # C1: region-C line additive; squared -> high-region cap

VALUES = np.array([0.0, 0.5, 1.0, 1.5, 2.0, 3.0, 4.0, 6.0], dtype=np.float32)
_ORD_LUT = np.clip(np.arange(256) - 3, 0, 7).astype(np.int32)
_Q_LUT = VALUES[_ORD_LUT]
# 4-bit pack decode: lo nibble = even element, hi nibble = odd element
_LO_ORD = np.clip((np.arange(256) & 15) - 3, 0, 7).astype(np.int32)
_HI_ORD = np.clip((np.arange(256) >> 4) - 3, 0, 7).astype(np.int32)
_LO_Q = VALUES[_LO_ORD]
_HI_Q = VALUES[_HI_ORD]

# ---------------------------------------------------------------- custom op
def _register_op(name, spec, subdim=False):
    if name in DO._SUB_OPCODE_FOR_NAME:          # idempotent across re-imports
        return next(op for op in DO.OPS if op.name == name)
    row = DO._CUSTOM_DVE_ROW_BASE + len(DO.OPS)
    shas = {}
    for ver in ("v3", "v4"):
        sc = DveOpSpec(name=name, opcode=row, uops=lower(spec, ver=ver),
                       rd1_en=_has_src1(spec))
        shas[ver] = sc.sha(ver)
    op = DO.DveOp(name, spec, subdim=subdim, uops_sha=shas)
    DO.OPS.append(op)
    DO._SUB_OPCODE_FOR_NAME[name] = row
    return op


def _absn(x):
    return Bin(AluOp.ABSOLUTE_VALUE, x, Zero)


def _fused_ref(in0, in1, s0, s1, imm2):
    a = np.abs(in0) + in1
    return np.minimum(a + np.minimum(a, s0),
                      np.maximum(a + s1, s1 * s1)).astype(np.float32)


_a = _absn(Src0) + Src1
P_FUSED = _register_op("BSR_FUSED_ORD_R", Spec(
    body=minn(_a + minn(_a, C0), maxx(_a + C1, sq(C1))),
    reference=_fused_ref,
))

# out[k] = min(in0[2k],15) + 16*min(in0[2k+1],15)  (via strided in0/in1 APs)
P_PACK = _register_op("BSR_PACK4", Spec(
    body=minn(Src0, C1) + minn(Src1, C1) * C0,
    reference=lambda in0, in1, s0, s1, imm2: (
        np.minimum(in0, s1) + np.minimum(in1, s1) * s0).astype(np.float32),
))

# ---------------------------------------------------------------- bass module
_NC_CACHE = {}


def _ap(t, offset, ap):
    return bass.AP(tensor=getattr(t, "tensor", t), offset=offset, ap=ap)


def build_nc():
    if "nc" in _NC_CACHE:
        return _NC_CACHE["nc"]
    nc = bacc.Bacc(None, target_bir_lowering=False)
    x = nc.dram_tensor("x", [SHARD_ELEMS], mybir.dt.float32, kind="ExternalInput")
    d = nc.dram_tensor("d", [SHARD_ELEMS // BLOCK], mybir.dt.float32,
                       kind="ExternalInput")
    o_elems = SHARD_ELEMS // 2 if PACK4 else SHARD_ELEMS
    o = nc.dram_tensor("o", [o_elems], mybir.dt.uint8, kind="ExternalOutput")

    DBMAX = FD // BLOCK
    NSLOT = 3
    xs = [nc.alloc_sbuf_tensor(f"xs{s}", [128, FD], mybir.dt.float32).ap()
          for s in range(NSLOT)]
    ds = [nc.alloc_sbuf_tensor(f"ds{s}", [128, DBMAX], mybir.dt.float32).ap()
          for s in range(NSLOT)]
    os_ = [nc.alloc_sbuf_tensor(f"os{s}", [128, FD], mybir.dt.uint8).ap()
           for s in range(NSLOT)]
    ps = [nc.alloc_sbuf_tensor(f"ps{s}", [128, FD // 2], mybir.dt.uint8).ap()
          for s in range(NSLOT)] if PACK4 else os_

    offs = [0]
    for f in CHUNK_FDS:
        offs.append(offs[-1] + 128 * f)
    NCH = len(CHUNK_FDS)

    # Per-slot load/store sems: DMA completions from different chunks land
    # out of order, so one shared counter would release a consumer while the
    # current chunk's transfer is still in flight. Within one slot, chunks
    # are NSLOT apart and the pipeline (wsem gates) guarantees ordering.
    with (
        nc.semaphore("ldsem0") as ldsem0,
        nc.semaphore("ldsem1") as ldsem1,
        nc.semaphore("ldsem2") as ldsem2,
        nc.semaphore("stsem0") as stsem0,
        nc.semaphore("stsem1") as stsem1,
        nc.semaphore("stsem2") as stsem2,
        nc.semaphore("fsem") as fsem,     # fused-op completions (frees xs/ds)
        nc.semaphore("wsem") as wsem,     # chunk-done (last DVE op) completions
        nc.Block() as block,
    ):
        ldsem = [ldsem0, ldsem1, ldsem2]
        stsem = [stsem0, stsem1, stsem2]
        xfree = fsem if PACK4 else wsem

        @block.sync
        def _(sync):
            # loads only: a store's wsem wait must never block load issue
            for i in range(NCH):
                s = i % NSLOT
                fd = CHUNK_FDS[i]
                db = fd // BLOCK
                if i >= NSLOT:
                    # slot's previous op consumed xs/ds
                    sync.wait_ge(xfree, i - NSLOT + 1)
                sync.dma_start(
                    out=xs[s][:, :fd],
                    in_=_ap(x, offs[i], [[fd, 128], [1, fd]]),
                ).then_inc(ldsem[s], 16)
                sync.dma_start(
                    out=ds[s][:, :db],
                    in_=_ap(d, offs[i] // BLOCK, [[db, 128], [1, db]]),
                ).then_inc(ldsem[s], 16)

        @block.gpsimd
        def _(gpsimd):
            # stores on their own engine queue (gpsimd SWDGE), gated only by
            # op completions. Measured faster than the Act HWDGE ring here.
            for j in range(NCH):
                s = j % NSLOT
                fd = CHUNK_FDS[j]
                gpsimd.wait_ge(wsem, j + 1)
                if PACK4:
                    gpsimd.dma_start(
                        out=_ap(o, offs[j] // 2, [[fd // 2, 128], [1, fd // 2]]),
                        in_=ps[s][:, :fd // 2],
                    ).then_inc(stsem[s], 16)
                else:
                    gpsimd.dma_start(
                        out=_ap(o, offs[j], [[fd, 128], [1, fd]]),
                        in_=os_[s][:, :fd],
                    ).then_inc(stsem[s], 16)
            for s in range(NSLOT):
                n_st = len([i for i in range(NCH) if i % NSLOT == s])
                gpsimd.wait_ge(stsem[s], 16 * n_st)

        @block.vector
        def _(vector):
            for i in range(NCH):
                s = i % NSLOT
                fd = CHUNK_FDS[i]
                db = fd // BLOCK
                vector.wait_ge(ldsem[s], 32 * (i // NSLOT + 1))
                if not PACK4 and i >= NSLOT:
                    # slot's previous store must have drained os_
                    vector.wait_ge(stsem[s], 16 * (i // NSLOT))
                fused = nc.vector._custom_dve(
                    P_FUSED,
                    out=_ap(os_[s], 0, [os_[s].ap[0], [BLOCK, db], [1, BLOCK]]),
                    in0=_ap(xs[s], 0, [xs[s].ap[0], [BLOCK, db], [1, BLOCK]]),
                    in1=_ap(ds[s], 0, [ds[s].ap[0], [1, db], [0, BLOCK]]),
                    s0=S_MIN, s1=S_ADD,
                )
                if PACK4:
                    fused.then_inc(fsem, 1)
                    if i >= NSLOT:
                        # slot's previous store must have drained ps
                        vector.wait_ge(stsem[s], 16 * (i // NSLOT))
                    nc.vector._custom_dve(
                        P_PACK,
                        out=ps[s][:, :fd // 2],
                        in0=_ap(os_[s], 0, [os_[s].ap[0], [2, fd // 2]]),
                        in1=_ap(os_[s], 1, [os_[s].ap[0], [2, fd // 2]]),
                        s0=16.0, s1=15.0,
                    ).then_inc(wsem, 1)
                else:
                    fused.then_inc(wsem, 1)

    nc.compile()
    _NC_CACHE["nc"] = nc
    return nc


# ---------------------------------------------------------------- host entry
def _delta_device(delta_raw):
    """0.5*tanh + D_OFFSET on the default jax backend — tanh bit-matches the
    oracle's eager computation (backend tanh differs from libm)."""
    import jax.numpy as jnp
    return np.asarray(0.5 * jnp.tanh(jnp.asarray(np.asarray(delta_raw)))
                      + np.float32(D_OFFSET))


def _install_trace_shim():
    """Optional: register the axon NTFF profiling hook so _trace=True works
    in containers whose antenv lacks axon_hooks. No-op on failure."""
    import sys, types
    if "antenv.axon_hooks" in sys.modules:
        return
    try:
        from trn_agent_boot.trn_boot import _ntff_profile_via_ctypes
        hook = _ntff_profile_via_ctypes("/opt/axon/libaxon_pjrt.so")
        mod = types.ModuleType("antenv.axon_hooks")
        mod.get_axon_ntff_profile_hook = lambda: hook
        mod.set_axon_ntff_profile_hook = lambda h: None
        sys.modules["antenv.axon_hooks"] = mod
    except Exception:
        pass


def kernel(x_scaled, delta_raw, _trace=False):
    if _trace:
        _install_trace_shim()
    x_scaled = np.ascontiguousarray(np.asarray(x_scaled), dtype=np.float32)
    delta = _delta_device(delta_raw).astype(np.float32, copy=False)

    nc = build_nc()
    in_maps = []
    for c in range(N_CORES):
        xsh = x_scaled[c * SHARD_ROWS:(c + 1) * SHARD_ROWS].reshape(-1)
        dsh = delta[c * (SHARD_ELEMS // BLOCK):(c + 1) * (SHARD_ELEMS // BLOCK)]
        in_maps.append({"x": xsh, "d": np.ascontiguousarray(dsh)})

    res = run_bass_kernel_spmd(nc, in_maps, list(range(N_CORES)), trace=_trace)

    ob = np.concatenate([res.results[c]["o"] for c in range(N_CORES)])
    if PACK4:
        o = np.empty(ROWS * COLS, dtype=np.int32)
        q = np.empty(ROWS * COLS, dtype=np.float32)
        o[0::2] = _LO_ORD[ob]
        o[1::2] = _HI_ORD[ob]
        q[0::2] = _LO_Q[ob]
        q[1::2] = _HI_Q[ob]
        o = o.reshape(ROWS, COLS)
        q = q.reshape(ROWS, COLS)
    else:
        o = _ORD_LUT[ob].reshape(ROWS, COLS)
        q = _Q_LUT[ob].reshape(ROWS, COLS)
    out = (q, o)
    if _trace:
        return out, res
    return out


# revision 25
# speedup vs baseline: 1.1552x; 1.0010x over previous
"""nn_BlockSharedRounding Trainium2 kernel.

Computes the forward of the block-shared soft rounding reference:
    a   = |x| + 0.5*tanh(delta_raw) per 32-block
    ord = searchsorted(BOUNDS, a, 'left')
    q   = VALUES[ord]                       (== abs_mix forward value)

Strategy: data-parallel over 8 NeuronCores (rows of x). Per core, a raw
Bass kernel streams [128, fd] fp32 chunks through ONE fused custom DVE
op that evaluates the e2m1 ordinal as the (round-to-nearest) uint8
conversion of a min/max of three linear functions of
a' = |x| + d' (d' = 0.5*tanh(delta)+1.5):

    byte = rne( min( a' + min(a', 3.5), max(a' + 3.0, 3.0^2) ) )

which equals ord+3 for ord in 0..7 (bytes >= 10 encode ord 7; bytes <= 3
encode ord 0).  The host decodes both outputs from the single uint8
stream with 256-entry LUTs: ord = clip(byte-3, 0, 7), q = VALUES[ord].

Per-core HBM traffic is 16.5 MiB read + 4 MiB write (vs 29.9 MB for the
4-pass predecessor), and the vector engine does a single 8-stage pass
(34 us) instead of four, leaving the kernel DMA-bound at ~60 us.

The fused op only has two scalar slots (the d' broadcast occupies Src1
in TTSS encoding), so the high-region cap constant is derived in the
hoisted seed stage as SQUARE(C1): 3.0^2 = 9.0, which lies in the valid
cap window [9, 9.5) for byte base B=3 under round-to-nearest output
conversion.

Pipeline: NSLOT=3 rotating SBUF slots; x+d loads issued from the sync
engine (HWDGE), stores from the gpsimd queue so a store's wait can
never block load issue; big chunks first so the final op+store tail is
tiny. Measured 68.7-69.0 us on trn2 (baseline 161 us).
"""
import numpy as np

import concourse.bass as bass
import concourse.bacc as bacc
import concourse.mybir as mybir
import concourse.dve_ops as DO
from concourse.dve_uop import DveOpSpec
from concourse.dve_spec import (
    Spec, Src0, Src1, C0, C1, Zero, Bin, AluOp, lower, minn, maxx, sq,
    _has_src1,
)
from concourse.bass_utils import run_bass_kernel_spmd

# ---------------------------------------------------------------- constants
N_CORES = 8
ROWS, COLS = 4096, 8192
SHARD_ROWS = ROWS // N_CORES            # 512
SHARD_ELEMS = SHARD_ROWS * COLS         # 4,194,304
BLOCK = 32
FD = 8192                               # max free dim per chunk
CHUNK_FDS = [8192, 8192, 8192, 4096, 2048, 1536, 512]
assert sum(CHUNK_FDS) * 128 == SHARD_ELEMS
PACK4 = False                           # pack two 4-bit ordinals per byte

# rne-mode design: byte = ord + 3, d' offset 1.5
D_OFFSET = 1.5
S_MIN = 3.5         # C0: region-B line constant (min operand)
S_ADD = 3.0         # C1: region-C line additive; squared -> high-region cap

VALUES = np.array([0.0, 0.5, 1.0, 1.5, 2.0, 3.0, 4.0, 6.0], dtype=np.float32)
_ORD_LUT = np.clip(np.arange(256) - 3, 0, 7).astype(np.int32)
_Q_LUT = VALUES[_ORD_LUT]
# 4-bit pack decode: lo nibble = even element, hi nibble = odd element
_LO_ORD = np.clip((np.arange(256) & 15) - 3, 0, 7).astype(np.int32)
_HI_ORD = np.clip((np.arange(256) >> 4) - 3, 0, 7).astype(np.int32)
_LO_Q = VALUES[_LO_ORD]
_HI_Q = VALUES[_HI_ORD]

# ---------------------------------------------------------------- custom op
def _register_op(name, spec, subdim=False):
    if name in DO._SUB_OPCODE_FOR_NAME:          # idempotent across re-imports
        return next(op for op in DO.OPS if op.name == name)
    row = DO._CUSTOM_DVE_ROW_BASE + len(DO.OPS)
    shas = {}
    for ver in ("v3", "v4"):
        sc = DveOpSpec(name=name, opcode=row, uops=lower(spec, ver=ver),
                       rd1_en=_has_src1(spec))
        shas[ver] = sc.sha(ver)
    op = DO.DveOp(name, spec, subdim=subdim, uops_sha=shas)
    DO.OPS.append(op)
    DO._SUB_OPCODE_FOR_NAME[name] = row
    return op


def _absn(x):
    return Bin(AluOp.ABSOLUTE_VALUE, x, Zero)


def _fused_ref(in0, in1, s0, s1, imm2):
    a = np.abs(in0) + in1
    return np.minimum(a + np.minimum(a, s0),
                      np.maximum(a + s1, s1 * s1)).astype(np.float32)


_a = _absn(Src0) + Src1
P_FUSED = _register_op("BSR_FUSED_ORD_R", Spec(
    body=minn(_a + minn(_a, C0), maxx(_a + C1, sq(C1))),
    reference=_fused_ref,
))

# out[k] = min(in0[2k],15) + 16*min(in0[2k+1],15)  (via strided in0/in1 APs)
P_PACK = _register_op("BSR_PACK4", Spec(
    body=minn(Src0, C1) + minn(Src1, C1) * C0,
    reference=lambda in0, in1, s0, s1, imm2: (
        np.minimum(in0, s1) + np.minimum(in1, s1) * s0).astype(np.float32),
))

# ---------------------------------------------------------------- bass module
_NC_CACHE = {}


def _ap(t, offset, ap):
    return bass.AP(tensor=getattr(t, "tensor", t), offset=offset, ap=ap)


def build_nc():
    if "nc" in _NC_CACHE:
        return _NC_CACHE["nc"]
    nc = bacc.Bacc(None, target_bir_lowering=False)
    x = nc.dram_tensor("x", [SHARD_ELEMS], mybir.dt.float32, kind="ExternalInput")
    d = nc.dram_tensor("d", [SHARD_ELEMS // BLOCK], mybir.dt.float32,
                       kind="ExternalInput")
    o_elems = SHARD_ELEMS // 2 if PACK4 else SHARD_ELEMS
    o = nc.dram_tensor("o", [o_elems], mybir.dt.uint8, kind="ExternalOutput")

    DBMAX = FD // BLOCK
    NSLOT = 3
    xs = [nc.alloc_sbuf_tensor(f"xs{s}", [128, FD], mybir.dt.float32).ap()
          for s in range(NSLOT)]
    ds = [nc.alloc_sbuf_tensor(f"ds{s}", [128, DBMAX], mybir.dt.float32).ap()
          for s in range(NSLOT)]
    os_ = [nc.alloc_sbuf_tensor(f"os{s}", [128, FD], mybir.dt.uint8).ap()
           for s in range(NSLOT)]
    ps = [nc.alloc_sbuf_tensor(f"ps{s}", [128, FD // 2], mybir.dt.uint8).ap()
          for s in range(NSLOT)] if PACK4 else os_

    offs = [0]
    for f in CHUNK_FDS:
        offs.append(offs[-1] + 128 * f)
    NCH = len(CHUNK_FDS)

    # Per-slot load/store sems: DMA completions from different chunks land
    # out of order, so one shared counter would release a consumer while the
    # current chunk's transfer is still in flight. Within one slot, chunks
    # are NSLOT apart and the pipeline (wsem gates) guarantees ordering.
    with (
        nc.semaphore("ldsem0") as ldsem0,
        nc.semaphore("ldsem1") as ldsem1,
        nc.semaphore("ldsem2") as ldsem2,
        nc.semaphore("stsem0") as stsem0,
        nc.semaphore("stsem1") as stsem1,
        nc.semaphore("stsem2") as stsem2,
        nc.semaphore("fsem") as fsem,     # fused-op completions (frees xs/ds)
        nc.semaphore("wsem") as wsem,     # chunk-done (last DVE op) completions
        nc.Block() as block,
    ):
        ldsem = [ldsem0, ldsem1, ldsem2]
        stsem = [stsem0, stsem1, stsem2]
        xfree = fsem if PACK4 else wsem

        @block.sync
        def _(sync):
            # loads only: a store's wsem wait must never block load issue
            for i in range(NCH):
                s = i % NSLOT
                fd = CHUNK_FDS[i]
                db = fd // BLOCK
                if i >= NSLOT:
                    # slot's previous op consumed xs/ds
                    sync.wait_ge(xfree, i - NSLOT + 1)
                sync.dma_start(
                    out=xs[s][:, :fd],
                    in_=_ap(x, offs[i], [[fd, 128], [1, fd]]),
                ).then_inc(ldsem[s], 16)
                sync.dma_start(
                    out=ds[s][:, :db],
                    in_=_ap(d, offs[i] // BLOCK, [[db, 128], [1, db]]),
                ).then_inc(ldsem[s], 16)

        @block.gpsimd
        def _(gpsimd):
            # stores on their own engine queue (gpsimd SWDGE), gated only by
            # op completions. Measured faster than the Act HWDGE ring here.
            for j in range(NCH):
                s = j % NSLOT
                fd = CHUNK_FDS[j]
                gpsimd.wait_ge(wsem, j + 1)
                if PACK4:
                    gpsimd.dma_start(
                        out=_ap(o, offs[j] // 2, [[fd // 2, 128], [1, fd // 2]]),
                        in_=ps[s][:, :fd // 2],
                    ).then_inc(stsem[s], 16)
                else:
                    gpsimd.dma_start(
                        out=_ap(o, offs[j], [[fd, 128], [1, fd]]),
                        in_=os_[s][:, :fd],
                    ).then_inc(stsem[s], 16)
            for s in range(NSLOT):
                n_st = len([i for i in range(NCH) if i % NSLOT == s])
                gpsimd.wait_ge(stsem[s], 16 * n_st)

        @block.vector
        def _(vector):
            for i in range(NCH):
                s = i % NSLOT
                fd = CHUNK_FDS[i]
                db = fd // BLOCK
                vector.wait_ge(ldsem[s], 32 * (i // NSLOT + 1))
                if not PACK4 and i >= NSLOT:
                    # slot's previous store must have drained os_
                    vector.wait_ge(stsem[s], 16 * (i // NSLOT))
                fused = nc.vector._custom_dve(
                    P_FUSED,
                    out=_ap(os_[s], 0, [os_[s].ap[0], [BLOCK, db], [1, BLOCK]]),
                    in0=_ap(xs[s], 0, [xs[s].ap[0], [BLOCK, db], [1, BLOCK]]),
                    in1=_ap(ds[s], 0, [ds[s].ap[0], [1, db], [0, BLOCK]]),
                    s0=S_MIN, s1=S_ADD,
                )
                if PACK4:
                    fused.then_inc(fsem, 1)
                    if i >= NSLOT:
                        # slot's previous store must have drained ps
                        vector.wait_ge(stsem[s], 16 * (i // NSLOT))
                    nc.vector._custom_dve(
                        P_PACK,
                        out=ps[s][:, :fd // 2],
                        in0=_ap(os_[s], 0, [os_[s].ap[0], [2, fd // 2]]),
                        in1=_ap(os_[s], 1, [os_[s].ap[0], [2, fd // 2]]),
                        s0=16.0, s1=15.0,
                    ).then_inc(wsem, 1)
                else:
                    fused.then_inc(wsem, 1)

    nc.compile()
    _NC_CACHE["nc"] = nc
    return nc


# ---------------------------------------------------------------- host entry
def _delta_device(delta_raw):
    """0.5*tanh + D_OFFSET on the default jax backend — tanh bit-matches the
    oracle's eager computation (backend tanh differs from libm)."""
    import jax.numpy as jnp
    return np.asarray(0.5 * jnp.tanh(jnp.asarray(np.asarray(delta_raw)))
                      + np.float32(D_OFFSET))


def _install_trace_shim():
    """Optional: register the axon NTFF profiling hook so _trace=True works
    in containers whose antenv lacks axon_hooks. No-op on failure."""
    import sys, types
    if "antenv.axon_hooks" in sys.modules:
        return
    try:
        from trn_agent_boot.trn_boot import _ntff_profile_via_ctypes
        hook = _ntff_profile_via_ctypes("/opt/axon/libaxon_pjrt.so")
        mod = types.ModuleType("antenv.axon_hooks")
        mod.get_axon_ntff_profile_hook = lambda: hook
        mod.set_axon_ntff_profile_hook = lambda h: None
        sys.modules["antenv.axon_hooks"] = mod
    except Exception:
        pass


def kernel(x_scaled, delta_raw, _trace=False):
    if _trace:
        _install_trace_shim()
    x_scaled = np.ascontiguousarray(np.asarray(x_scaled), dtype=np.float32)
    delta = _delta_device(delta_raw).astype(np.float32, copy=False)

    nc = build_nc()
    in_maps = []
    for c in range(N_CORES):
        xsh = x_scaled[c * SHARD_ROWS:(c + 1) * SHARD_ROWS].reshape(-1)
        dsh = delta[c * (SHARD_ELEMS // BLOCK):(c + 1) * (SHARD_ELEMS // BLOCK)]
        in_maps.append({"x": xsh, "d": np.ascontiguousarray(dsh)})

    res = run_bass_kernel_spmd(nc, in_maps, list(range(N_CORES)), trace=_trace)

    ob = np.concatenate([res.results[c]["o"] for c in range(N_CORES)])
    if PACK4:
        o = np.empty(ROWS * COLS, dtype=np.int32)
        q = np.empty(ROWS * COLS, dtype=np.float32)
        o[0::2] = _LO_ORD[ob]
        o[1::2] = _HI_ORD[ob]
        q[0::2] = _LO_Q[ob]
        q[1::2] = _HI_Q[ob]
        o = o.reshape(ROWS, COLS)
        q = q.reshape(ROWS, COLS)
    else:
        o = _ORD_LUT[ob].reshape(ROWS, COLS)
        q = _Q_LUT[ob].reshape(ROWS, COLS)
    out = (q, o)
    if _trace:
        return out, res
    return out
